# revision 18
# baseline (speedup 1.0000x reference)
"""Trainium2 Bass kernel for softmax(user_emb @ id_emb.T, axis=-1).

Shapes (hardcoded): user_emb [8192, 1024] f32, id_emb [8192, 1024] f32,
out [8192, 8192] f32.

Sharding: user_emb rows split across 8 NeuronCores (1024 rows each),
id_emb replicated; each core computes its [1024, 8192] score block and
row-softmax independently; outputs concatenated on axis 0.

Per-core kernel: 3-pass fp16 hi/lo split matmul (near-fp32 accuracy at
1 cycle/row per pass on the PE vs 4 cycles/row for native fp32):
    S = Uh @ Eh.T + Ul @ Eh.T + Uh @ El.T   (Ul@El term ~2^-24, dropped)
The contraction dim (d) must sit on SBUF partitions for both operands.
Both operands are loaded transposed straight from DRAM with strided DMA
access patterns (the DRAM-side reads stay 512-byte contiguous: one j-row
spread over 128 d-partitions), then split into fp16 hi/lo on chip.
E is streamed twice (two 512-row m-blocks); the [512, 8192] f32 score
block stays in SBUF where the row softmax runs fused: reduce_max(negate)
-> in-place Exp with accum_out row sums -> reciprocal -> in-place scale
-> output DMA.
"""
import numpy as np

P = 128          # partitions
D = 1024         # embedding dim (contraction)
SEQ = 8192       # id_emb rows (softmax axis)
ROWS = 1024      # user rows per core
NCORES = 8
KT = D // P      # 8 contraction chunks
NW = 512         # matmul moving free dim (one PSUM bank of f32)
NT = SEQ // NW   # 16 n-tiles
MT = ROWS // P   # 8 m-tiles per core
MB = 4           # m-tiles per E-sweep
NSWEEP = (MT + MB - 1) // MB
KH = KT // 2     # k-chunks per load slab

_CACHE = {}


def _build(reps=1):
    import concourse.tile as tile
    from concourse import bacc, mybir

    F32 = mybir.dt.float32
    F16 = mybir.dt.float16
    EXP = mybir.ActivationFunctionType.Exp
    AX = mybir.AxisListType.X
    MAX = mybir.AluOpType.max

    nc = bacc.Bacc("TRN2", target_bir_lowering=False, debug=False,
                   num_devices=NCORES)
    u = nc.dram_tensor("u", [ROWS, D], F32, kind="ExternalInput").ap()
    e = nc.dram_tensor("e", [SEQ, D], F32, kind="ExternalInput").ap()
    o = nc.dram_tensor("o", [ROWS, SEQ], F32, kind="ExternalOutput").ap()

    with tile.TileContext(nc) as tc:
        with (
            tc.tile_pool(name="ut", bufs=1) as utp,
            tc.tile_pool(name="sblk", bufs=MB) as sp,
            tc.tile_pool(name="stage", bufs=2) as stp,
            tc.tile_pool(name="et", bufs=2) as etp,
            tc.tile_pool(name="stats", bufs=2 * MB) as statp,
            tc.tile_pool(name="pss", bufs=8, space="PSUM") as pss,
        ):
            for rep in range(reps):
                for s in range(NSWEEP):
                    m_lo = s * MB
                    m_hi = min(m_lo + MB, MT)
                    nmb = m_hi - m_lo
                    # UT layout [P(d), KT*MB*P], columns k*(MB*P) + ml*P + i
                    ut_h = utp.tile([P, KT * MB * P], F16, tag="ut_h",
                                    name=f"ut_h_{rep}_{s}")
                    ut_l = utp.tile([P, KT * MB * P], F16, tag="ut_l",
                                    name=f"ut_l_{rep}_{s}")
                    for h in range(KT // KH):
                        st = stp.tile([P, KH * MB * P], F32, tag="stage")
                        for kk in range(KH):
                            k = h * KH + kk
                            src = u[m_lo * P:m_hi * P, k * P:(k + 1) * P]
                            nc.sync.dma_start(
                                st[:, kk * MB * P:(kk + 1) * MB * P],
                                src.rearrange("i p -> p i"))
                        c0 = h * KH * MB * P
                        c1 = (h + 1) * KH * MB * P
                        nc.scalar.copy(ut_h[:, c0:c1], st[:])
                        nc.vector.tensor_sub(ut_l[:, c0:c1], st[:],
                                             ut_h[:, c0:c1])
                    stiles = [sp.tile([P, SEQ], F32, tag="sblk",
                                      name=f"s_{rep}_{s}_{i}")
                              for i in range(nmb)]
                    mxblks = [statp.tile([P, NT], F32, tag="mxblk",
                                         name=f"mx_{rep}_{s}_{i}")
                              for i in range(nmb)]
                    for n in range(NT):
                        # transposed strided load of one [NW, D] slab of E,
                        # then fp16 hi/lo split.
                        # ET layout [P(d), KT*NW], columns k*NW + j.
                        et_h = etp.tile([P, KT * NW], F16, tag="et_h")
                        et_l = etp.tile([P, KT * NW], F16, tag="et_l")
                        for h in range(KT // KH):
                            st = stp.tile([P, KH * NW], F32, tag="stage")
                            for kk in range(KH):
                                k = h * KH + kk
                                src = e[n * NW:(n + 1) * NW,
                                        k * P:(k + 1) * P]
                                nc.sync.dma_start(
                                    st[:, kk * NW:(kk + 1) * NW],
                                    src.rearrange("j p -> p j"))
                            c0 = h * KH * NW
                            c1 = (h + 1) * KH * NW
                            nc.scalar.copy(et_h[:, c0:c1], st[:])
                            nc.vector.tensor_sub(et_l[:, c0:c1], st[:],
                                                 et_h[:, c0:c1])
                        # matmuls: 3 passes x KT chunks per m-tile
                        for ml in range(nmb):
                            acc = pss.tile([P, NW], F32, tag="pss")
                            n_mm = 3 * KT
                            i_mm = 0
                            for k in range(KT):
                                kb = k * MB * P
                                uh = ut_h[:, kb + ml * P:kb + (ml + 1) * P]
                                ul = ut_l[:, kb + ml * P:kb + (ml + 1) * P]
                                eh = et_h[:, k * NW:(k + 1) * NW]
                                el = et_l[:, k * NW:(k + 1) * NW]
                                for lhsT, rhs in ((uh, eh), (ul, eh),
                                                  (uh, el)):
                                    nc.tensor.matmul(
                                        acc[:], lhsT, rhs,
                                        start=(i_mm == 0),
                                        stop=(i_mm == n_mm - 1))
                                    i_mm += 1
                            dst = stiles[ml][:, n * NW:(n + 1) * NW]
                            if (n + ml) % 2:
                                nc.vector.tensor_copy(dst, acc[:])
                            else:
                                nc.scalar.copy(dst, acc[:])
                            # incremental row max of this n-tile
                            nc.vector.tensor_reduce(
                                mxblks[ml][:, n:n + 1], dst, axis=AX, op=MAX)
                    # fused row softmax on each finished [P, SEQ] block
                    for ml in range(nmb):
                        m = m_lo + ml
                        stile = stiles[ml]
                        negmx = statp.tile([P, 1], F32, tag="negmx")
                        nc.vector.tensor_reduce(negmx[:], mxblks[ml][:],
                                                axis=AX, op=MAX, negate=True)
                        sm = statp.tile([P, 1], F32, tag="sm")
                        nc.scalar.activation(stile[:], stile[:], EXP,
                                             bias=negmx[:], scale=1.0,
                                             accum_out=sm[:])
                        rcp = statp.tile([P, 1], F32, tag="rcp")
                        nc.vector.reciprocal(rcp[:], sm[:])
                        nc.vector.tensor_scalar_mul(stile[:], stile[:],
                                                    rcp[:])
                        nc.sync.dma_start(o[m * P:(m + 1) * P, :], stile[:])
    nc.compile()
    return nc


def _get_nc(reps=1):
    if reps not in _CACHE:
        _CACHE[reps] = _build(reps)
    return _CACHE[reps]


_LAST_PHASES = {}


def _run_spmd(nc, user_emb, id_emb):
    """Execute the SPMD kernel via PJRT/shard_map with id_emb replicated
    (one 32 MB transfer instead of eight) and user_emb sharded on axis 0.

    Mirrors concourse.bass2jax.run_bass_via_pjrt's multi-core path, minus
    the per-core input concatenation."""
    import jax
    import numpy as _np
    from jax.sharding import Mesh, PartitionSpec
    from jax.experimental.shard_map import shard_map
    from concourse import bass2jax, mybir

    bass2jax.install_neuronx_cc_hook()
    assert nc.dbg_addr is None
    partition_name = (nc.partition_id_tensor.name
                      if nc.partition_id_tensor else None)

    in_names, out_names, out_avals, zero_outs = [], [], [], []
    for alloc in nc.m.functions[0].allocations:
        if not isinstance(alloc, mybir.MemoryLocationSet):
            continue
        name = alloc.memorylocations[0].name
        if alloc.kind == "ExternalInput":
            if name != partition_name:
                in_names.append(name)
        elif alloc.kind == "ExternalOutput":
            out_names.append(name)
            shape = tuple(alloc.tensor_shape)
            dtype = mybir.dt.np(alloc.dtype)
            out_avals.append(jax.core.ShapedArray(shape, dtype))
            zero_outs.append(
                _np.zeros((NCORES * shape[0], *shape[1:]), dtype))
    n_params = len(in_names)
    in_names = in_names + out_names
    if partition_name is not None:
        in_names.append(partition_name)

    def _body(*args):
        operands = list(args)
        if partition_name is not None:
            operands.append(bass2jax.partition_id_tensor())
        outs = bass2jax._bass_exec_p.bind(
            *operands,
            out_avals=tuple(out_avals),
            in_names=tuple(in_names),
            out_names=tuple(out_names),
            lowering_input_output_aliases=(),
            sim_require_finite=True,
            sim_require_nnan=True,
            nc=nc,
        )
        return tuple(outs)

    devices = jax.devices()[:NCORES]
    mesh = Mesh(_np.asarray(devices), ("core",))
    by_name = {"u": PartitionSpec("core"), "e": PartitionSpec()}
    in_specs = tuple(by_name[n] for n in in_names[:n_params]) + (
        PartitionSpec("core"),) * len(out_names)
    out_specs = (PartitionSpec("core"),) * len(out_names)
    sharded = jax.jit(
        shard_map(_body, mesh=mesh, in_specs=in_specs, out_specs=out_specs,
                  check_rep=False),
        donate_argnums=tuple(range(n_params, n_params + len(out_names))),
        keep_unused=True,
    )

    import os
    import time as _time
    from concurrent.futures import ThreadPoolExecutor
    import jax.numpy as jnp
    from jax.sharding import NamedSharding

    prof = os.environ.get("KERNEL_PROFILE")
    ins = {"u": user_emb, "e": id_emb}
    t0 = _time.time()
    args = [jax.device_put(ins[n]) for n in in_names[:n_params]]
    # allocate the donated output buffers on-device (no host->device bytes)
    shardings = [NamedSharding(mesh, PartitionSpec("core"))] * len(zero_outs)
    mkzeros = jax.jit(
        lambda: tuple(jnp.zeros(z.shape, z.dtype) for z in zero_outs),
        out_shardings=tuple(shardings))
    dz = mkzeros()
    jax.block_until_ready((args, dz))
    t1 = _time.time()
    out_arrs = sharded(*args, *dz)
    jax.block_until_ready(out_arrs)
    t2 = _time.time()
    # fetch output shards in parallel (one stream per device)
    out = out_arrs[0]
    res = _np.empty(out.shape, out.dtype)
    shards = sorted(out.addressable_shards, key=lambda s: s.index[0].start or 0)

    def _pull(sh):
        res[sh.index] = _np.asarray(sh.data)

    with ThreadPoolExecutor(max_workers=8) as ex:
        list(ex.map(_pull, shards))
    t3 = _time.time()
    _LAST_PHASES.update(upload=t1 - t0, exec=t2 - t1, fetch=t3 - t2)
    if prof:
        print(f"[kernel] upload={t1-t0:.2f}s exec={t2-t1:.2f}s "
              f"fetch={t3-t2:.2f}s", flush=True)
    return res


def kernel(user_emb: np.ndarray, id_emb: np.ndarray) -> np.ndarray:
    nc = _get_nc()
    user_emb = np.ascontiguousarray(user_emb, dtype=np.float32)
    id_emb = np.ascontiguousarray(id_emb, dtype=np.float32)
    return _run_spmd(nc, user_emb, id_emb)
